# revision 26
# baseline (speedup 1.0000x reference)
"""CWFA_AO kernel for 8x TRN2 NeuronCores (Bass/Tile).

Math: per (n,t) transition matrix W[n,t][(i,l)] = sum_jk A[i,j,k,l] a[n,t,j] o[n,t,k],
formed as one big PE matmul  W[nt, (il)] = M[jk, nt]^T @ Atilde[jk, (il)]  in
float32r (full-rate fp32 on the PE), with the a x o outer-product operand M built
on GPSIMD from PE-encoded a/o (encoders folded into PE matmuls via host-replicated
weight layouts).  The T=128 recurrence runs as two 64-tick vector chains (forward
from alpha, backward from Omega) on the DVE using all 128 lanes: each (dir, traj)
pair occupies two lanes (contraction split in half), one [128,512] broadcast-
multiply + one strided reduce per tick; a small PE matmul (hsum weights) sums the
two half-lane partials and hops the state into the next tick's PSUM slot.

Sharding: data-parallel over N (32 trajectories per core), replicated weights.
"""

import numpy as np

N, T = 256, 128
DRAW = 16
DA = DO = 32
R = 32
NCORES = 8
NL = N // NCORES          # 32 trajectories per core
TH = T // 2               # 64 ticks per direction
NT = NL * T               # 4096 columns per core
FWD_COLS = NL * TH        # 2048
STRIP = 512
F32 = np.float32

_CACHE = {}


def _build_bass(stage=4):
    import concourse.bass as bass
    import concourse.bacc as bacc
    import concourse.mybir as mybir
    import concourse.tile as tile
    from contextlib import ExitStack

    fp32 = mybir.dt.float32
    fp32r = mybir.dt.float32r
    mult = mybir.AluOpType.mult
    add = mybir.AluOpType.add
    AX = mybir.AxisListType.X
    try:
        ACT_COPY = mybir.ActivationFunctionType.Copy
    except AttributeError:
        ACT_COPY = mybir.ActivationFunctionType.Identity

    nc = bacc.Bacc()
    es = ExitStack()

    # ---- DRAM I/O ----
    d_actT = nc.dram_tensor("actT", [17, NT], fp32r, kind="ExternalInput")
    d_obsT = nc.dram_tensor("obsT", [17, NT], fp32r, kind="ExternalInput")
    d_wo4 = nc.dram_tensor("wo4", [17, 128], fp32r, kind="ExternalInput")
    d_wasel = nc.dram_tensor("wasel", [17, 1024], fp32r, kind="ExternalInput")
    d_ail = nc.dram_tensor("ail", [1024, 1024], fp32r, kind="ExternalInput")
    d_ali = nc.dram_tensor("ali", [1024, 1024], fp32r, kind="ExternalInput")
    d_init = nc.dram_tensor("init0", [128, 16], fp32, kind="ExternalInput")
    d_eye = nc.dram_tensor("eye64", [128, 64], fp32, kind="ExternalInput")
    d_hsum = nc.dram_tensor("hsum", [128, 64], fp32, kind="ExternalInput")
    d_sfin = nc.dram_tensor("sfin", [128, 64], fp32, kind="ExternalInput")
    d_out = nc.dram_tensor("out", [32], fp32, kind="ExternalOutput")

    def ap(t, off, dims):
        return bass.AP(t[:].tensor, off, dims)

    with tile.TileContext(nc) as tc:
        with (
            tc.tile_pool(name="consts", bufs=1) as cpool,
            tc.tile_pool(name="work", bufs=1) as wpool,
            tc.tile_pool(name="enc", bufs=2) as epool,
            tc.tile_pool(name="mst", bufs=3) as mpool,
            tc.tile_pool(name="st", bufs=6) as stpool,
            tc.tile_pool(name="pp", bufs=2) as ppool,
            tc.tile_pool(name="wev", bufs=3) as wevpool,
            tc.tile_pool(name="pe", bufs=1, space="PSUM") as pse,
            tc.tile_pool(name="pw", bufs=2, space="PSUM") as psw,
            tc.tile_pool(name="psml", bufs=1, space="PSUM") as psml,
        ):
            # ---- constant loads ----
            actT = cpool.tile([17, NT], fp32r, tag="actT")
            obsT = cpool.tile([17, NT], fp32r, tag="obsT")
            wo4 = cpool.tile([17, 128], fp32r, tag="wo4")
            wasel = cpool.tile([17, 1024], fp32r, tag="wasel")
            eye = cpool.tile([128, 64], fp32, tag="eye")
            hsum = cpool.tile([128, 64], fp32, tag="hsum")
            sfin = cpool.tile([128, 64], fp32, tag="sfin")
            init0 = cpool.tile([128, 16], fp32, tag="init0")
            nc.sync.dma_start(actT[:], d_actT[:])
            nc.sync.dma_start(obsT[:], d_obsT[:])
            nc.sync.dma_start(wo4[:], d_wo4[:])
            nc.sync.dma_start(wasel[:], d_wasel[:])
            nc.sync.dma_start(eye[:], d_eye[:])
            nc.sync.dma_start(hsum[:], d_hsum[:])
            nc.sync.dma_start(sfin[:], d_sfin[:])
            nc.sync.dma_start(init0[:], d_init[:])
            ail = []
            ali = []
            for c in range(8):
                ta = cpool.tile([128, 1024], fp32r, tag=f"ail{c}")
                tb = cpool.tile([128, 1024], fp32r, tag=f"ali{c}")
                nc.sync.dma_start(ta[:], d_ail[128 * c:128 * c + 128, :])
                nc.sync.dma_start(tb[:], d_ali[128 * c:128 * c + 128, :])
                ail.append(ta)
                ali.append(tb)

            # ---- persistent state ----
            # state_ps cols [0:16) = even ticks, [16:32) = odd ticks;
            # partition (h, dir, n) holds v[dir, n, 16h:16h+16]
            state_ps = psml.tile([128, 32], fp32, tag="state_ps")
            fin_ps = psml.tile([128, 64], fp32, tag="fin_ps")
            bfin = wpool.tile([128, 32], fp32, tag="bfin")
            junk = wpool.tile([128, 32], fp32, tag="junk")
            res = wpool.tile([128, 1], fp32, tag="res")

            # state init -> PSUM cols [0:16)
            nc.tensor.matmul(state_ps[0:64, 0:16], eye[0:64, :],
                             init0[0:64, :], start=True, stop=True,
                             skip_group_check=True)
            nc.tensor.matmul(state_ps[64:128, 0:16], eye[64:128, :],
                             init0[64:128, :], start=True, stop=True,
                             skip_group_check=True)

            mstrips = {}      # strip -> list of 8 M tiles [128, 512]

            def encoder_strip(u, meng=None):
                meng = meng or nc.gpsimd
                lo = STRIP * u
                po = pse.tile([128, STRIP], fp32, tag="po")
                nc.tensor.matmul(po[:], wo4[:], obsT[:, lo:lo + STRIP],
                                 start=True, stop=True)
                o4 = epool.tile([128, STRIP], fp32r, tag="o4")
                nc.scalar.activation(o4[:], po[:], ACT_COPY)
                mts = []
                for c in range(8):
                    pa = pse.tile([128, STRIP], fp32, tag="pa")
                    nc.tensor.matmul(pa[:],
                                     wasel[:, 128 * c:128 * c + 128],
                                     actT[:, lo:lo + STRIP],
                                     start=True, stop=True)
                    a32 = epool.tile([128, STRIP], fp32r, tag="a32")
                    nc.scalar.activation(a32[:], pa[:], ACT_COPY)
                    mt = mpool.tile([128, STRIP], fp32r, tag=f"m{c}")
                    meng.tensor_tensor(mt[:], a32[:], o4[:], mult)
                    mts.append(mt)
                mstrips[u] = mts

            def form_piece(wps, mt, msub, amat_c, c):
                # one jk-chunk's contribution to a W nt-chunk (2 MMs)
                lhs = mt[:, 128 * msub:128 * msub + 128]
                nc.tensor.matmul(wps[:, 0:512], lhs, amat_c[:, 0:512],
                                 start=(c == 0), stop=(c == 7))
                nc.tensor.matmul(wps[:, 512:1024], lhs,
                                 amat_c[:, 512:1024],
                                 start=(c == 0), stop=(c == 7))

            def tick(tau, st):
                e = 16 * (tau % 2)
                prod = ppool.tile([128, 512], fp32, tag="prod")
                partials = ppool.tile([128, 32], fp32, tag="partials")
                # prod[(h,d,n), x, y] = W * v[x]-bcast   (x: 16 of 32, y: 32)
                nc.vector.tensor_tensor(
                    ap(prod, 0, [[512, 128], [32, 16], [1, 32]]),
                    ap(st, 0, [[512, 128], [32, 16], [1, 32]]),
                    ap(state_ps, e, [[32, 128], [1, 16], [0, 32]]),
                    mult)
                # partials[(h,d,n), y] = sum_x prod
                nc.vector.tensor_reduce(
                    ap(partials, 0, [[32, 128], [1, 32]]),
                    ap(prod, 0, [[512, 128], [1, 32], [32, 16]]),
                    AX, add)
                if tau < TH - 1:
                    e2 = 16 * ((tau + 1) % 2)
                    # state'[(h,d,n), :] = sum_h' partials[(h',d,n), y-half h]
                    nc.tensor.matmul(state_ps[0:64, e2:e2 + 16], hsum[:],
                                     partials[:, 0:16], start=True, stop=True,
                                     skip_group_check=True)
                    nc.tensor.matmul(state_ps[64:128, e2:e2 + 16], hsum[:],
                                     partials[:, 16:32], start=True, stop=True,
                                     skip_group_check=True)
                return partials

            last_partials = [None]

            def emit_one_tick(t2, wf, wb):
                tq = t2 % 4
                st = stpool.tile([128, 512], fp32, tag="st")
                r0 = 32 * tq
                nc.sync.dma_start(st[0:32, :], wf[r0:r0 + 32, 0:512])
                nc.sync.dma_start(st[32:64, :], wb[r0:r0 + 32, 0:512])
                nc.sync.dma_start(st[64:96, :], wf[r0:r0 + 32, 512:1024])
                nc.sync.dma_start(st[96:128, :], wb[r0:r0 + 32, 512:1024])
                last_partials[0] = tick(t2, st)

            PIPE = 2
            pend = []
            for s in range((16 + PIPE) if stage >= 2 else 0):
                # tick emissions for pair s-PIPE, interleaved into the
                # formation stream so hop MMs never queue behind a full
                # 32-matmul block on the in-order PE
                tickq = []
                if s >= PIPE and stage >= 3:
                    ss, twf, twb = pend.pop(0)
                    tickq = [(4 * ss + q, twf, twb) for q in range(4)]

                def pop_tick():
                    if tickq:
                        tq, twf, twb = tickq.pop(0)
                        emit_one_tick(tq, twf, twb)

                if s < 16:
                    if s == 0:
                        encoder_strip(0, nc.vector)
                        encoder_strip(4, nc.vector)
                    if s % 4 == 0 and s < 12:
                        encoder_strip(s // 4 + 1)
                        encoder_strip(4 + s // 4 + 1)
                    uf, ub = s // 4, 4 + s // 4
                    wf_ps = psw.tile([128, 1024], fp32, tag="wps")
                    wb_ps = psw.tile([128, 1024], fp32, tag="wps")
                    wf = wevpool.tile([128, 1024], fp32, tag="wfs")
                    wb = wevpool.tile([128, 1024], fp32, tag="wbs")
                    for c in range(8):
                        form_piece(wf_ps, mstrips[uf][c], s % 4, ail[c], c)
                        if c % 2 == 1:
                            pop_tick()
                    nc.scalar.activation(wf[:], wf_ps[:], ACT_COPY)
                    for c in range(8):
                        form_piece(wb_ps, mstrips[ub][c], s % 4, ali[c], c)
                        if c % 2 == 1:
                            pop_tick()
                    nc.scalar.activation(wb[:], wb_ps[:], ACT_COPY)
                    pend.append((s, wf, wb))
                while tickq:
                    pop_tick()
            if stage >= 3:
                for ss, twf, twb in pend:
                    for q in range(4):
                        emit_one_tick(4 * ss + q, twf, twb)

            # ---- final: out[n] = sum_y v_f[n,y] * v_b[n,y] ----
            if stage >= 4:
                partials = last_partials[0]
                nc.tensor.matmul(fin_ps[0:32, 0:32], sfin[:, 0:32],
                                 partials[:], start=True, stop=True,
                                 skip_group_check=True)
                nc.tensor.matmul(fin_ps[0:32, 32:64], sfin[:, 32:64],
                                 partials[:], start=True, stop=True,
                                 skip_group_check=True)
                nc.scalar.activation(bfin[0:32, :], fin_ps[0:32, 32:64],
                                     ACT_COPY)
                nc.vector.tensor_tensor(junk[0:32, :], bfin[0:32, :],
                                        fin_ps[0:32, 0:32], mult)
                nc.vector.tensor_reduce(res[0:32, 0:1], junk[0:32, :],
                                        AX, add)
                nc.sync.dma_start(d_out[:], res[0:32, 0:1])
            else:
                nc.sync.dma_start(d_out[:], res[0:32, 0:1])

    es.close()
    nc.compile()
    return nc


def _prep_core(actions, obss):
    """actions/obss: [NL, T, 16] for one core -> [17, NT] column-permuted."""
    def enc(x):
        fwd = x[:, :TH, :].transpose(2, 1, 0).reshape(DRAW, FWD_COLS)
        bwd = x[:, :TH - 1:-1, :].transpose(2, 1, 0).reshape(DRAW, FWD_COLS)
        m = np.concatenate([fwd, bwd], axis=1)
        return np.concatenate([m, np.ones((1, NT), F32)], axis=0).astype(F32)
    return enc(actions), enc(obss)


def _consts(Wa, ba, Wo, bo, alpha, A, Omega):
    k4 = np.tile(np.arange(DO), 4)
    wo4 = np.concatenate([Wo[:, k4], bo[k4][None, :]], axis=0).astype(F32)
    cols = []
    for c in range(8):
        idx = np.repeat(np.arange(4 * c, 4 * c + 4), 32)
        cols.append(np.concatenate([Wa[:, idx], ba[idx][None, :]], axis=0))
    wasel = np.concatenate(cols, axis=1).astype(F32)
    ail = np.ascontiguousarray(A.transpose(1, 2, 0, 3).reshape(1024, 1024))
    ali = np.ascontiguousarray(A.transpose(1, 2, 3, 0).reshape(1024, 1024))
    seeds = [alpha, Omega[:, 0]]
    init0 = np.zeros((128, 16), F32)
    hsum = np.zeros((128, 64), F32)
    sfin = np.zeros((128, 64), F32)
    for h in range(2):
        for d in range(2):
            r = 64 * h + 32 * d
            init0[r:r + 32, :] = np.tile(seeds[d][16 * h:16 * h + 16], (32, 1))
            for n in range(32):
                hsum[r + n, 32 * d + n] = 1.0
                sfin[r + n, 32 * d + n] = 1.0
    eye64 = np.tile(np.eye(64, dtype=F32), (2, 1))
    return dict(wo4=wo4, wasel=wasel, ail=ail, ali=ali, init0=init0,
                eye64=eye64, hsum=hsum, sfin=sfin)


def kernel(actions, obss, Wa, ba, Wo, bo, alpha, A, Omega):
    actions = np.asarray(actions, F32)
    obss = np.asarray(obss, F32)
    Wa = np.asarray(Wa, F32); ba = np.asarray(ba, F32)
    Wo = np.asarray(Wo, F32); bo = np.asarray(bo, F32)
    alpha = np.asarray(alpha, F32)
    A = np.asarray(A, F32)
    Omega = np.asarray(Omega, F32)

    cst = _consts(Wa, ba, Wo, bo, alpha, A, Omega)
    in_maps = []
    for c in range(NCORES):
        at, ot = _prep_core(actions[NL * c:NL * c + NL],
                            obss[NL * c:NL * c + NL])
        in_maps.append({"actT": at, "obsT": ot, **cst})

    if "nc" not in _CACHE:
        _CACHE["nc"] = _build_bass()
    from concourse.bass_utils import run_bass_kernel_spmd
    r = run_bass_kernel_spmd(_CACHE["nc"], in_maps, list(range(NCORES)))
    outs = []
    for c in range(NCORES):
        o = r.results[c]["out"] if isinstance(r.results[c], dict) else r.results[c]
        outs.append(np.asarray(o, F32).reshape(NL))
    return np.concatenate(outs).astype(F32)
